# revision 6
# baseline (speedup 1.0000x reference)
"""Differential attention (B=2, S=2048, HS=1024, H=16, KV=4, D=64) on 8 trn2 cores.

Sharding: core c = (b, g) with b = c // 4 (data parallel on batch) and
g = c % 4 (tensor parallel over the 4 KV head groups; each core owns the
4 query heads of its group).  Each core computes its 4 heads' normed
attention output and a row-parallel partial of the output projection
(out_pt = (O_heads @ Wo_rows)^T); the host sums the 4 partials per batch.

Device pipeline per core (matmuls in fp32r ~= tf32), fully interleaved by
q-tile so projection, attention, RMSNorm and the output projection pipeline
across engines:
  per q-tile: xT slice -> Q^T/K^T/V^T projections (PE) with RoPE
  (partition-swap via SBUF->SBUF DMA, sign baked into the sin table);
  per (head, q-tile): flash-style causal attention:
    S^T[k,q] = K^T-strip.T @ Q^T-strip  (two 64-contraction row-strips),
    P = exp(S/8) on ACT (no row-max: scores are O(5), fp32 exp is safe),
    causal diagonal wedge zeroed by gpsimd affine_select on P,
    U^T[128,q] += [V|ones].T @ P  (ones block replicates the softmax
    denominator onto partitions 64..127),
    epilogue: lane-aligned reciprocal, SBUF->SBUF partition shift,
    O = U1/r1 - lam*U2/r2 (lam folded into V2, subtract on gpsimd),
    RMSNorm via ones-matmul row-sum of O^2 + ln/exp rsqrt (one ACT table),
  per q-tile: partial^T = Wo_rows.T @ O_norm^T -> DRAM.
subln_w is folded into Wo rows on the host.
"""

import math
import sys

import numpy as np

try:
    import concourse.bass as bass  # noqa: F401
except ImportError:
    sys.path.insert(0, "/opt/trn_rl_repo")

import concourse.bass as bass
import concourse.tile as tile
from concourse import bacc, mybir
from concourse import bass_utils

f32 = mybir.dt.float32
f32r = mybir.dt.float32r
bf16 = mybir.dt.bfloat16
AF = mybir.ActivationFunctionType
ALU = mybir.AluOpType

B, S, HS = 2, 2048, 1024
H, KV, D = 16, 4, 64
NHL = 4            # query heads per core
NQT = 4            # q tiles of 512
QTW = 512
NKT = 16           # k tiles of 128
NHS = 8            # hs tiles of 128
NEG = -1e9
EPS = 1e-5

_prog_cache = {}


def _build_program(lam: float):
    nc = bacc.Bacc("TRN2", target_bir_lowering=False, debug=False,
                   enable_asserts=False, num_devices=8)

    xt = nc.dram_tensor("xt", [HS, S], f32r, kind="ExternalInput").ap()
    wq = nc.dram_tensor("wq", [HS, 512], f32r, kind="ExternalInput").ap()
    wk = nc.dram_tensor("wk", [HS, 128], f32r, kind="ExternalInput").ap()
    wv = nc.dram_tensor("wv", [HS, 64], f32r, kind="ExternalInput").ap()
    wo = nc.dram_tensor("wo", [256, HS], f32r, kind="ExternalInput").ap()
    cos_t = nc.dram_tensor("cos_t", [128, S], f32, kind="ExternalInput").ap()
    sin_t = nc.dram_tensor("sin_t", [128, S], f32, kind="ExternalInput").ap()
    idf = nc.dram_tensor("idf", [64, 64], f32, kind="ExternalInput").ap()
    ones = nc.dram_tensor("ones", [128, 64], f32r, kind="ExternalInput").ap()
    out_pt = nc.dram_tensor("out_pt", [HS, S], f32, kind="ExternalOutput").ap()

    with tile.TileContext(nc) as tc:
        with tc.tile_pool(name="persist", bufs=1) as pp, \
             tc.tile_pool(name="loc", bufs=2) as loc, \
             tc.tile_pool(name="pwk", bufs=2) as pwk, \
             tc.tile_pool(name="patt", bufs=3) as pa, \
             tc.tile_pool(name="ep", bufs=2) as pe, \
             tc.tile_pool(name="rmsp", bufs=2) as prm, \
             tc.psum_pool(name="ps", bufs=2) as ps_:

            ones_sb = pp.tile([128, 64], f32r, name="ones", tag="ones")
            nc.sync.dma_start(ones_sb[:], ones[:])
            idf_sb = pp.tile([64, 64], f32, name="idf", tag="idf")
            nc.sync.dma_start(idf_sb[:], idf[:])
            wo_sb = []
            for t in range(2):
                w = pp.tile([128, HS], f32r, name=f"wo{t}", tag=f"wo{t}")
                nc.sync.dma_start(w[:], wo[t * 128:(t + 1) * 128, :])
                wo_sb.append(w)
            wq_sb, wk_sb, wv_sb = [], [], []
            for hs in range(NHS):
                t_ = pp.tile([128, 512], f32r, name=f"wq{hs}", tag=f"wq{hs}")
                nc.sync.dma_start(t_[:], wq[hs * 128:(hs + 1) * 128, :])
                wq_sb.append(t_)
                t_ = pp.tile([128, 128], f32r, name=f"wk{hs}", tag=f"wk{hs}")
                nc.sync.dma_start(t_[:], wk[hs * 128:(hs + 1) * 128, :])
                wk_sb.append(t_)
                t_ = pp.tile([128, 64], f32r, name=f"wv{hs}", tag=f"wv{hs}")
                nc.sync.dma_start(t_[:], wv[hs * 128:(hs + 1) * 128, :])
                wv_sb.append(t_)
            k_sb = pp.tile([128, S], f32r, name="k", tag="k")
            va = [pp.tile([128, 128], f32r, name=f"va{kt}", tag=f"va{kt}")
                  for kt in range(NKT)]
            vb = [pp.tile([128, 128], f32r, name=f"vb{kt}", tag=f"vb{kt}")
                  for kt in range(NKT)]
            for kt in range(NKT):
                nc.vector.tensor_copy(va[kt][:, 64:128], ones_sb[:])
                nc.vector.tensor_copy(vb[kt][:, 64:128], ones_sb[:])
            eps_sb = pp.tile([1, 1], f32, name="eps", tag="eps")
            nc.vector.memset(eps_sb[:], EPS)

            def rope_block(ps, dst, cosq, sinq):
                # dst = ps * cos + swap32(ps) * sin   (sign baked in sin)
                qpl = pwk.tile([128, QTW], f32, name="qpl", tag="qpl")
                nc.vector.tensor_copy(qpl[:], ps[:])
                qsw = pwk.tile([128, QTW], f32, name="qsw", tag="qsw")
                for blk, src in ((0, 32), (1, 0), (2, 96), (3, 64)):
                    nc.sync.dma_start(qsw[blk * 32:(blk + 1) * 32, :],
                                      qpl[src:src + 32, :])
                qc = pwk.tile([128, QTW], f32, name="qc", tag="qc")
                nc.vector.tensor_mul(qc[:], qpl[:], cosq[:])
                qs = pwk.tile([128, QTW], f32, name="qs", tag="qs")
                nc.vector.tensor_mul(qs[:], qsw[:], sinq[:])
                nc.vector.tensor_add(dst, qc[:], qs[:])

            for qt in range(NQT):
                qlo, qhi = qt * QTW, (qt + 1) * QTW
                # ---- projections + RoPE for this q tile ----
                xt_sb = []
                for hs in range(NHS):
                    t_ = pwk.tile([128, QTW], f32r, name=f"xt{hs}", tag=f"xt{hs}")
                    nc.sync.dma_start(t_[:], xt[hs * 128:(hs + 1) * 128, qlo:qhi])
                    xt_sb.append(t_)
                cosq = loc.tile([128, QTW], f32, name="cosq", tag="cosq")
                nc.sync.dma_start(cosq[:], cos_t[:, qlo:qhi])
                sinq = loc.tile([128, QTW], f32, name="sinq", tag="sinq")
                nc.sync.dma_start(sinq[:], sin_t[:, qlo:qhi])
                qloc = [loc.tile([128, QTW], f32r, name=f"q{j}", tag=f"q{j}")
                        for j in range(NHL)]
                for j in range(NHL):
                    psq = ps_.tile([128, QTW], f32, name="psq", tag="big")
                    for hs in range(NHS):
                        nc.tensor.matmul(
                            psq[:], wq_sb[hs][:, j * 128:(j + 1) * 128],
                            xt_sb[hs][:], start=(hs == 0), stop=(hs == NHS - 1))
                    rope_block(psq, qloc[j][:], cosq, sinq)
                psk = ps_.tile([128, QTW], f32, name="psk", tag="big")
                for hs in range(NHS):
                    nc.tensor.matmul(psk[:], wk_sb[hs][:], xt_sb[hs][:],
                                     start=(hs == 0), stop=(hs == NHS - 1))
                rope_block(psk, k_sb[:, qlo:qhi], cosq, sinq)
                psv = ps_.tile([64, QTW], f32, name="psv", tag="big")
                for hs in range(NHS):
                    nc.tensor.matmul(psv[:], wv_sb[hs][:], xt_sb[hs][:],
                                     start=(hs == 0), stop=(hs == NHS - 1))
                vtq = loc.tile([64, QTW], f32, name="vtq", tag="vtq")
                nc.vector.tensor_copy(vtq[:], psv[:])
                for kk in range(4):
                    kt = 4 * qt + kk
                    psvt = ps_.tile([128, 64], f32, name="psvt", tag="big")
                    nc.tensor.transpose(psvt[:], vtq[:, kk * 128:(kk + 1) * 128],
                                        idf_sb[:])
                    nc.vector.tensor_copy(va[kt][:, 0:64], psvt[:])
                    nc.vector.tensor_scalar_mul(vb[kt][:, 0:64], psvt[:], lam)

                # ---- attention + rmsnorm per head ----
                opair = [loc.tile([128, QTW], f32, name=f"op{t}", tag=f"op{t}")
                         for t in range(2)]
                onq = [loc.tile([128, QTW], f32r, name=f"on{t}", tag=f"on{t}")
                       for t in range(2)]
                for j in range(NHL):
                    half, pt = (j % 2) * 64, j // 2
                    last_kt = 4 * qt + 3
                    psu = ps_.tile([128, 2 * QTW], f32, name="psu", tag="psU")
                    for kt in range(last_kt + 1):
                        jd = kt - 4 * qt
                        q0 = 128 * jd if jd >= 0 else 0
                        diag = jd >= 0
                        pss = ps_.tile([128, 2 * QTW], f32, name="pss", tag="big")
                        nc.tensor.matmul(
                            pss[:, q0:QTW],
                            k_sb[0:64, kt * 128:(kt + 1) * 128],
                            qloc[j][0:64, q0:QTW],
                            start=True, stop=True, skip_group_check=True)
                        nc.tensor.matmul(
                            pss[:, QTW + q0:2 * QTW],
                            k_sb[64:128, kt * 128:(kt + 1) * 128],
                            qloc[j][64:128, q0:QTW],
                            start=True, stop=True, skip_group_check=True)
                        p12 = pa.tile([128, 2 * QTW], f32r, name="p12", tag="p12")
                        nc.scalar.activation(p12[:, q0:2 * QTW], pss[:, q0:2 * QTW],
                                             AF.Exp, scale=0.125)
                        if diag:
                            for off in (q0, QTW + q0):
                                nc.gpsimd.affine_select(
                                    p12[:, off:off + 128], p12[:, off:off + 128],
                                    pattern=[[1, 128]], compare_op=ALU.is_ge,
                                    fill=0.0, base=0, channel_multiplier=-1)
                        nc.tensor.matmul(
                            psu[:, q0:QTW], va[kt][:], p12[:, q0:QTW],
                            start=(kt == 0), stop=(kt == last_kt),
                            skip_group_check=True)
                        nc.tensor.matmul(
                            psu[:, QTW + q0:2 * QTW], vb[kt][:],
                            p12[:, QTW + q0:2 * QTW],
                            start=(kt == 0), stop=(kt == last_kt),
                            skip_group_check=True)
                    # epilogue: O^T = U1/r1 - lam*U2/r2
                    wri = pe.tile([128, 2 * QTW], f32, name="wri", tag="wri")
                    nc.vector.reciprocal(wri[64:128, :], psu[64:128, :])
                    nc.sync.dma_start(wri[0:64, :], wri[64:128, :])
                    t1 = pe.tile([64, QTW], f32, name="t1", tag="t1")
                    nc.vector.tensor_mul(t1[:], psu[0:64, 0:QTW], wri[0:64, 0:QTW])
                    t2 = pe.tile([64, QTW], f32, name="t2", tag="t2")
                    nc.vector.tensor_mul(t2[:], psu[0:64, QTW:2 * QTW],
                                         wri[0:64, QTW:2 * QTW])
                    if j % 2 == 0:
                        nc.gpsimd.tensor_sub(opair[pt][0:64, :], t1[:], t2[:])
                    else:
                        otmp = pe.tile([64, QTW], f32, name="otmp", tag="otmp")
                        nc.gpsimd.tensor_sub(otmp[:], t1[:], t2[:])
                        nc.sync.dma_start(opair[pt][64:128, :], otmp[:])
                    # rmsnorm for this (head, q tile)
                    osq = prm.tile([128, QTW], f32r, name="osq", tag="osq")
                    nc.vector.tensor_mul(osq[half:half + 64, :],
                                         opair[pt][half:half + 64, :],
                                         opair[pt][half:half + 64, :])
                    psss = ps_.tile([1, QTW], f32, name="psss", tag="big")
                    nc.tensor.matmul(psss[:], ones_sb[half:half + 64, 0:1],
                                     osq[half:half + 64, :], start=True, stop=True)
                    lnq = prm.tile([1, QTW], f32, name="lnq", tag="lnq")
                    nc.scalar.activation(lnq[:], psss[:], AF.Ln, scale=1.0 / 64.0,
                                         bias=eps_sb[0:1, 0:1])
                    rmq = prm.tile([1, QTW], f32, name="rmq", tag="rmq")
                    nc.scalar.activation(rmq[:], lnq[:], AF.Exp, scale=-0.5)
                    rsb = prm.tile([128, QTW], f32, name="rsb", tag="rsb")
                    nc.gpsimd.partition_broadcast(rsb[:], rmq[0:1, :])
                    nc.vector.tensor_mul(onq[pt][half:half + 64, :],
                                         opair[pt][half:half + 64, :],
                                         rsb[half:half + 64, :])

                # ---- output projection for this q tile ----
                for oc in range(8):
                    psw = ps_.tile([128, QTW], f32, name="psw", tag="big")
                    nc.tensor.matmul(psw[:], wo_sb[0][:, oc * 128:(oc + 1) * 128],
                                     onq[0][:], start=True, stop=False)
                    nc.tensor.matmul(psw[:], wo_sb[1][:, oc * 128:(oc + 1) * 128],
                                     onq[1][:], start=False, stop=True)
                    ow = prm.tile([128, QTW], f32, name="ow", tag="ow")
                    if oc % 2 == 0:
                        nc.scalar.copy(ow[:], psw[:])
                    else:
                        nc.vector.tensor_copy(ow[:], psw[:])
                    nc.sync.dma_start(out_pt[oc * 128:(oc + 1) * 128, qlo:qhi],
                                      ow[:])

    nc.compile()
    return nc


def get_program(lam: float):
    key = round(float(lam), 9)
    if key not in _prog_cache:
        _prog_cache[key] = _build_program(float(lam))
    return _prog_cache[key]


def ml_bf16():
    import ml_dtypes
    return ml_dtypes.bfloat16


def _host_inputs(x, rope_cos, rope_sin, Wq, Wk, Wv, Wo, subln_w, lam):
    cos_t = np.ascontiguousarray(np.tile(rope_cos.T, (4, 1))).astype(np.float32)
    sin64 = np.concatenate([-rope_sin.T, rope_sin.T], axis=0)
    sin_t = np.ascontiguousarray(np.tile(sin64, (2, 1))).astype(np.float32)
    idf = np.eye(64, dtype=np.float32)
    ones = np.ones((128, 64), np.float32)
    sub4 = np.tile(subln_w.astype(np.float32), 4)[:, None]

    in_maps = []
    for c in range(8):
        b, g = c // 4, c % 4
        xtc = np.ascontiguousarray(x[b].T).astype(np.float32)
        cols = []
        for j in range(NHL):
            h = 4 * g + j
            cols.append(Wq[:, h * 64:(h + 1) * 64])
            cols.append(Wq[:, (H + h) * 64:(H + h + 1) * 64])
        wq_c = np.ascontiguousarray(np.concatenate(cols, axis=1)).astype(np.float32)
        wk_c = np.ascontiguousarray(np.concatenate(
            [Wk[:, g * 64:(g + 1) * 64], Wk[:, (KV + g) * 64:(KV + g + 1) * 64]],
            axis=1)).astype(np.float32)
        wv_c = np.ascontiguousarray(Wv[:, g * 64:(g + 1) * 64]).astype(np.float32)
        wo_c = np.ascontiguousarray(
            Wo[g * 256:(g + 1) * 256, :] * sub4).astype(np.float32)
        in_maps.append({
            "xt": xtc, "wq": wq_c, "wk": wk_c, "wv": wv_c, "wo": wo_c,
            "cos_t": cos_t, "sin_t": sin_t, "idf": idf, "ones": ones,
        })
    return in_maps


def _compute_lam(lambda_q1, lambda_k1, lambda_q2, lambda_k2):
    li = 0.8 - 0.6 * math.exp(-0.3)
    l1 = np.exp(np.dot(lambda_q1.astype(np.float32), lambda_k1.astype(np.float32)))
    l2 = np.exp(np.dot(lambda_q2.astype(np.float32), lambda_k2.astype(np.float32)))
    return float(l1 - l2 + li)


def _numpy_reference(x, rope_cos, rope_sin, attention_mask, Wq, Wk, Wv, Wo,
                     lambda_q1, lambda_k1, lambda_q2, lambda_k2, subln_w):
    """Pure-numpy fallback, only used if the mask is not the expected causal one."""
    bsz, seq_len, _ = x.shape

    def rope(t):
        c = np.concatenate([rope_cos, rope_cos], axis=-1)[None, None]
        s = np.concatenate([rope_sin, rope_sin], axis=-1)[None, None]
        t1, t2 = np.split(t, 2, axis=-1)
        rot = np.concatenate([-t2, t1], axis=-1)
        return t * c + rot * s

    q = (x @ Wq).reshape(bsz, seq_len, 2 * H, D)
    q1 = np.transpose(q[:, :, :H], (0, 2, 1, 3))
    q2 = np.transpose(q[:, :, H:], (0, 2, 1, 3))
    k = (x @ Wk).reshape(bsz, seq_len, 2 * KV, D)
    k1 = np.transpose(k[:, :, :KV], (0, 2, 1, 3))
    k2 = np.transpose(k[:, :, KV:], (0, 2, 1, 3))
    v = np.transpose((x @ Wv).reshape(bsz, seq_len, KV, D), (0, 2, 1, 3))
    q1, q2, k1, k2 = rope(q1), rope(q2), rope(k1), rope(k2)
    gr = H // KV
    k1 = np.repeat(k1, gr, axis=1)
    k2 = np.repeat(k2, gr, axis=1)
    v = np.repeat(v, gr, axis=1)
    scale = 1.0 / math.sqrt(D)

    def smax(a):
        a = a - a.max(axis=-1, keepdims=True)
        e = np.exp(a)
        return e / e.sum(axis=-1, keepdims=True)

    a1 = smax(np.einsum("bhqd,bhkd->bhqk", q1, k1) * scale + attention_mask)
    a2 = smax(np.einsum("bhqd,bhkd->bhqk", q2, k2) * scale + attention_mask)
    lam = _compute_lam(lambda_q1, lambda_k1, lambda_q2, lambda_k2)
    attn = a1 - lam * a2
    out = np.einsum("bhqk,bhkd->bhqd", attn, v)
    inv = 1.0 / np.sqrt(np.mean(out * out, axis=-1, keepdims=True) + EPS)
    out = out * inv * subln_w
    out = np.transpose(out, (0, 2, 1, 3)).reshape(bsz, seq_len, HS)
    return (out @ Wo).astype(np.float32)


LAST_RESULT = None


def kernel(x, rope_cos, rope_sin, attention_mask, Wq, Wk, Wv, Wo,
           lambda_q1, lambda_k1, lambda_q2, lambda_k2, subln_w):
    global LAST_RESULT
    x = np.asarray(x, np.float32)
    kk, qq = np.arange(S)[:, None], np.arange(S)[None, :]
    causal = np.where(qq <= kk, 0.0, NEG).astype(np.float32)[None, None]
    am = np.asarray(attention_mask, np.float32)
    if am.shape != (1, 1, S, S) or not np.array_equal(am, causal):
        return _numpy_reference(x, rope_cos, rope_sin, am, Wq, Wk, Wv, Wo,
                                lambda_q1, lambda_k1, lambda_q2, lambda_k2,
                                subln_w)

    lam = _compute_lam(lambda_q1, lambda_k1, lambda_q2, lambda_k2)
    nc = get_program(lam)
    in_maps = _host_inputs(x, np.asarray(rope_cos, np.float32),
                           np.asarray(rope_sin, np.float32),
                           np.asarray(Wq, np.float32), np.asarray(Wk, np.float32),
                           np.asarray(Wv, np.float32), np.asarray(Wo, np.float32),
                           np.asarray(subln_w, np.float32), lam)
    res = bass_utils.run_bass_kernel_spmd(nc, in_maps, core_ids=list(range(8)))
    LAST_RESULT = res
    y = np.zeros((B, S, HS), np.float32)
    for c in range(8):
        y[c // 4] += res.results[c]["out_pt"].T
    return y
